# revision 27
# baseline (speedup 1.0000x reference)
"""Causal self-attention (B=4, T=2048, C=1024, H=16) on 8 Trainium2 NeuronCores.

Sharding: core = (batch b = core//2, head-group g = core%2, 8 heads each).

Key ideas over the dense formulation:
  - Padded keys (~50% of tokens) are compacted away on the host: K/V
    projections, S = K^T.T Q^T, exp and PV run only over the ~1024 valid
    keys per batch (padded to TC, a multiple of 128). Causality on the
    compacted index is still a contiguous prefix per query, so the k-tile
    loop bounds shrink; ragged tile edges are handled by host-precomputed
    bf16 {0,1} masks multiplied into P after the exp (only over the ragged
    column span; pad rows are killed by zeroing their V rows + ones column).
  - All matmul operands are bf16 (PSUM accumulation stays fp32): halves
    DMA/SBUF and makes LDWEIGHTS cheap enough to hide behind matmuls.
  - All DRAM inputs are host-pre-transposed to partition-major [128, ...]
    layouts so DMA descriptors are large contiguous lines.
  - Q^T projection for q-block nt is computed during attention block nt-1
    (only nt=0 in the projection phase): the early attention blocks are
    exp-latency-bound, and the filler matmuls keep the PE busy so the HAM
    clock gate never drops back to 1.2GHz mid-kernel.
  - Softmax normalize: DVE copy+reciprocal of the PSUM denominator row,
    gpsimd broadcast, DVE multiplies; out-projection of a q-block issues
    right after its last head-pair normalizes.

Host: per-batch token compaction, transposes, bf16 casts, ragged masks,
sums the two head-group partials per batch and adds bproj.
"""

import os
import sys

for _p in ("/opt/trn_rl_repo",):
    if _p not in sys.path:
        sys.path.append(_p)

import numpy as np
import ml_dtypes

B, T, C = 4, 2048, 1024
H, D = 16, 64
HPC = 8          # heads per core
GC = HPC * D     # 512 channels per core
N_CORES = 8
P = 128
NT = T // 512    # 4 q-blocks of 512
MT = GC // 128   # 4 m-tiles (head pairs)
CT = C // 128    # 8 contraction tiles

BF = ml_dtypes.bfloat16
_cached = {}


def _attn_meta(pm):
    """Compile-time step structure shared by all cores (union over batches).

    Returns (TC, steps, NBP, pad_tiles): steps[qt] = tuple of
    (ktc, q0, qe, mask_idx); columns [q0, qe) of the tile get the ragged
    causal mask multiplied in (mask_idx == -1: tile fully valid, no mask).
    Columns >= qe are fully causal-valid in every batch; pad rows beyond the
    valid count are killed by zeroing their Vp rows (incl. the ones column),
    so they never need masking. pad_tiles lists k-tiles with pad slots in
    some batch."""
    idx = [np.nonzero(pm[b])[0] for b in range(B)]
    cnt = [len(i) for i in idx]
    TCT = -(-max(cnt) // P)
    TC = TCT * P
    INF = 1 << 30
    first = np.full((B, TCT), INF, np.int64)
    last = np.full((B, TCT), -1, np.int64)
    haspad = np.zeros((B, TCT), bool)
    for b in range(B):
        for t in range(TCT):
            lo, hi = t * P, min((t + 1) * P, cnt[b])
            if lo < cnt[b]:
                first[b, t] = idx[b][lo]
                last[b, t] = idx[b][hi - 1]
            haspad[b, t] = (t + 1) * P > cnt[b]
    steps = []
    nmask = 0
    for qt in range(NT):
        row = []
        for ktc in range(TCT):
            fmin = int(first[:, ktc].min())
            if fmin >= (qt + 1) * 512:
                continue
            q0 = max(0, fmin - qt * 512) & ~7
            lmax = int(last[:, ktc].max())
            if lmax > qt * 512:
                qe = min(512, (lmax - qt * 512 + 1 + 7) & ~7)
                mi = nmask
                nmask += 1
            else:
                qe = q0
                mi = -1
            row.append((ktc, q0, qe, mi))
        steps.append(tuple(row))
    pad_tiles = tuple(t for t in range(TCT) if haspad[:, t].any())
    return TC, tuple(steps), nmask, pad_tiles


def _build(TC, steps, NBP, pad_tiles):
    import concourse.tile as tile
    from concourse import bacc, mybir
    import concourse.bass as bass

    TCT = TC // P
    f32 = mybir.dt.float32
    bf = mybir.dt.bfloat16
    AF = mybir.ActivationFunctionType
    ADD = mybir.AluOpType.add
    MUL = mybir.AluOpType.mult
    NBIAS = 2 * MT + TCT  # packed bq | bk | valid

    nc = bacc.Bacc("TRN2", target_bir_lowering=False, debug=False)

    # all inputs partition-major, host-pre-transposed
    xT_d = nc.dram_tensor("xT", [P, CT, T], bf, kind="ExternalInput")
    xkv_d = nc.dram_tensor("xkv", [P, CT, TC], bf, kind="ExternalInput")
    wq_d = nc.dram_tensor("wq", [P, MT, CT, P], bf, kind="ExternalInput")
    wk_d = nc.dram_tensor("wk", [P, MT, CT, P], bf, kind="ExternalInput")
    wv_d = nc.dram_tensor("wv", [P, CT, GC], bf, kind="ExternalInput")
    wp_d = nc.dram_tensor("wp", [P, MT, C], bf, kind="ExternalInput")
    bb_d = nc.dram_tensor("bb", [P, NBIAS], f32, kind="ExternalInput")
    bv_d = nc.dram_tensor("bv", [GC], f32, kind="ExternalInput")
    msk_d = nc.dram_tensor("msk", [P, max(NBP, 1), 512], bf, kind="ExternalInput")
    out_d = nc.dram_tensor("out", [T, C], f32, kind="ExternalOutput")

    with tile.TileContext(nc) as tc:
        with tc.tile_pool(name="persist", bufs=1) as persist, \
             tc.tile_pool(name="ps", bufs=2, space="PSUM") as ps_pool, \
             tc.tile_pool(name="ppool", bufs=6) as ppool, \
             tc.tile_pool(name="ypool", bufs=2) as ypool, \
             tc.tile_pool(name="rpool", bufs=2) as rpool, \
             tc.tile_pool(name="bcpool", bufs=2) as bcpool, \
             tc.tile_pool(name="otpool", bufs=3) as otpool:
            QT = persist.tile([P, MT, T], bf, tag="QT")
            KT = persist.tile([P, MT, TC], bf, tag="KT")
            Vp = persist.tile([P, TCT, HPC, D + 1], bf, tag="Vp")
            MS = persist.tile([P, max(NBP, 1), 512], bf, tag="MS")
            xq_s = persist.tile([P, CT, T], bf, tag="xq")
            xkv_s = persist.tile([P, CT, TC], bf, tag="xkv")
            wq_s = persist.tile([P, MT, CT, P], bf, tag="wq")
            wk_s = persist.tile([P, MT, CT, P], bf, tag="wk")
            wv_s = persist.tile([P, CT, GC], bf, tag="wv")
            wp_s = persist.tile([P, MT, C], bf, tag="wp")
            bb_s = persist.tile([P, NBIAS], f32, tag="bb")
            bv_s = persist.tile([P, GC], f32, tag="bv")

            # --- DMAs, in the order compute consumes them ---------------
            nc.sync.dma_start(bb_s[:], bb_d[:])
            bv_ap = bass.AP(tensor=bv_d[:].tensor, offset=0, ap=[[0, P], [1, GC]])
            nc.sync.dma_start(bv_s[:], bv_ap)
            nc.sync.dma_start(wk_s[:, 0, :, :], wk_d[:, 0, :, :])
            nc.sync.dma_start(xkv_s[:, 0:2, :], xkv_d[:, 0:2, :])
            nc.sync.dma_start(wk_s[:, 1:MT, :, :], wk_d[:, 1:MT, :, :])
            for c2 in range(2, CT, 2):
                nc.sync.dma_start(xkv_s[:, c2:c2 + 2, :], xkv_d[:, c2:c2 + 2, :])
            for c2 in range(0, CT, 2):
                nc.sync.dma_start(wv_s[:, c2:c2 + 2, :], wv_d[:, c2:c2 + 2, :])
            for c2 in range(0, CT, 2):
                nc.sync.dma_start(xq_s[:, c2:c2 + 2, :], xT_d[:, c2:c2 + 2, :])
            nc.sync.dma_start(wq_s[:], wq_d[:])
            if NBP:
                nc.sync.dma_start(MS[:], msk_d[:])
            nc.sync.dma_start(wp_s[:], wp_d[:])

            # ones column of Vp (softmax denominator accumulator row)
            nc.vector.memset(Vp[:, :, :, D:D + 1], 1.0)

            # matmul PSUM output is capped at one bank = 512 fp32/partition
            k_chunks = [(c0, min(c0 + 512, TC)) for c0 in range(0, TC, 512)]

            # ---------------- K^T projection ----------------------------
            for m in range(MT):
                for (c0, c1) in k_chunks:
                    sl = slice(c0, c1)
                    pk = ps_pool.tile([P, 512], f32, tag="SS", name="pk")
                    for c in range(CT):
                        nc.tensor.matmul(
                            pk[:, 0:c1 - c0], wk_s[:, m, c, :],
                            xkv_s[:, c, sl],
                            start=(c == 0), stop=(c == CT - 1))
                    nc.vector.tensor_scalar(
                        out=KT[:, m, sl], in0=pk[:, 0:c1 - c0],
                        scalar1=bb_s[:, MT + m:MT + m + 1], scalar2=None,
                        op0=ADD)

            # ---------------- V projection ------------------------------
            for tt in range(TCT):
                pv = ps_pool.tile([P, GC], f32, tag="SS", name="pv")
                for c in range(CT):
                    nc.tensor.matmul(
                        pv[:], xkv_s[:, c, tt * P:(tt + 1) * P], wv_s[:, c, :],
                        start=(c == 0), stop=(c == CT - 1))
                nc.vector.tensor_add(
                    Vp[:, tt, :, 0:D],
                    pv[:].rearrange("p (h d) -> p h d", h=HPC),
                    bv_s[:].rearrange("p (h d) -> p h d", h=HPC))
                if tt in pad_tiles:
                    # zero V rows + ones column of pad slots: they then
                    # contribute nothing to numerator or denominator
                    nc.vector.tensor_scalar(
                        out=Vp[:, tt, :, :], in0=Vp[:, tt, :, :],
                        scalar1=bb_s[:, 2 * MT + tt:2 * MT + tt + 1],
                        scalar2=None, op0=MUL)

            # ------------- Q^T projection (scaled by 1/8) ---------------
            def q_proj_tile(m, nt):
                sl = slice(nt * 512, (nt + 1) * 512)
                pq = ps_pool.tile([P, 512], f32, tag="SS", name="pq")
                for c in range(CT):
                    nc.tensor.matmul(
                        pq[:], wq_s[:, m, c, :], xq_s[:, c, sl],
                        start=(c == 0), stop=(c == CT - 1))
                nc.vector.tensor_scalar(
                    out=QT[:, m, sl], in0=pq[:],
                    scalar1=bb_s[:, m:m + 1], scalar2=0.125,
                    op0=ADD, op1=MUL)

            for m in range(MT):
                q_proj_tile(m, 0)  # nt >= 1 interleaved into attention

            # ---------------- attention + out-projection ----------------
            def proj_piece(yT_, tt, ts):
                for nh in range(2):
                    pp_ = ps_pool.tile([P, 512], f32, tag="OO")
                    for cj in range(MT):
                        nc.tensor.matmul(
                            pp_[:], yT_[:, cj, ts * P:(ts + 1) * P],
                            wp_s[:, cj, nh * 512:(nh + 1) * 512],
                            start=(cj == 0), stop=(cj == MT - 1))
                    ot = otpool.tile([P, 512], f32, tag="ot")
                    if nh == 0:
                        nc.vector.tensor_copy(ot[:], pp_[:])
                    else:
                        nc.scalar.activation(ot[:], pp_[:], AF.Copy)
                    nc.sync.dma_start(
                        out_d[tt * P:(tt + 1) * P, nh * 512:(nh + 1) * 512], ot[:])

            for qt in range(NT):
                yTq = ypool.tile([P, MT, 512], bf, tag="yTq")
                srow = steps[qt]
                ns = len(srow)
                OO_map = {}
                pend = []
                LAG = 3

                def normalize_and_aux(j_, qt=qt, yTq=yTq):
                    OO_ = OO_map[j_]
                    lraw = rpool.tile([1, 2, 512], f32, tag="lraw")
                    nc.vector.tensor_copy(lraw[0:1, :, :], OO_[D:D + 1, :, :])
                    rec = rpool.tile([1, 2, 512], f32, tag="rec")
                    nc.vector.reciprocal_approx_fast(
                        rec[0:1, :, :], lraw[0:1, :, :])
                    bc = bcpool.tile([P, 2, 512], f32, tag="bc")
                    nc.gpsimd.partition_broadcast(bc[:], rec[0:1, :, :], channels=P)
                    nc.vector.tensor_mul(yTq[0:D, j_, :], OO_[0:D, 0, :], bc[0:D, 0, :])
                    nc.vector.tensor_mul(yTq[D:P, j_, :], OO_[0:D, 1, :], bc[D:P, 1, :])
                    if j_ == MT - 1:
                        # all 4 head-pairs of this q-block normalized:
                        # project and store the block
                        for ts in range(4):
                            proj_piece(yTq, qt * 4 + ts, ts)

                def emit_pv(entry):
                    j_, si_, z_, PP_ = entry
                    OO_ = OO_map[j_]
                    ktc = srow[si_][0]
                    last_ = (si_ == ns - 1)
                    for e in range(2):
                        nc.tensor.matmul(
                            OO_[:, e, z_:512], Vp[:, ktc, 2 * j_ + e, :],
                            PP_[:, e, z_:512],
                            start=(si_ == 0), stop=last_)
                    if last_:
                        normalize_and_aux(j_)

                for j in range(MT):
                    OO_map[j] = ps_pool.tile(
                        [D + 1, 2, 512], f32, tag="OO", name="OO")
                    for si, (ktc, q0, qe, mi) in enumerate(srow):
                        SS = ps_pool.tile([P, 2, 512], f32, tag="SS")
                        nc.tensor.matmul(
                            SS[:, 0, q0:512], KT[0:D, j, ktc * P:(ktc + 1) * P],
                            QT[0:D, j, qt * 512 + q0:(qt + 1) * 512],
                            start=True, stop=True)
                        nc.tensor.matmul(
                            SS[:, 1, q0:512], KT[D:P, j, ktc * P:(ktc + 1) * P],
                            QT[D:P, j, qt * 512 + q0:(qt + 1) * 512],
                            start=True, stop=True)
                        PP = ppool.tile([P, 2, 512], bf, tag="PP")
                        nc.scalar.activation(
                            PP[:, :, q0:512], SS[:, :, q0:512], AF.Exp)
                        if mi >= 0:
                            tm = MS[:, mi, q0:qe]
                            mask_b = bass.AP(
                                tensor=tm.tensor, offset=tm.offset,
                                ap=[list(tm.ap[0]), [0, 2], list(tm.ap[1])])
                            nc.vector.tensor_mul(
                                PP[:, :, q0:qe], PP[:, :, q0:qe], mask_b)
                        pend.append((j, si, q0, PP))
                        if len(pend) > LAG:
                            emit_pv(pend.pop(0))
                    if qt < NT - 1:
                        # PE filler while exp drains: Q^T proj for the next
                        # q-block, one m-tile per head-pair iteration
                        q_proj_tile(j, qt + 1)
                while pend:
                    emit_pv(pend.pop(0))

    nc.compile()
    return nc


def _get_nc(TC, steps, NBP, pad_tiles):
    key = (TC, steps, pad_tiles)
    if key not in _cached:
        _cached[key] = _build(TC, steps, NBP, pad_tiles)
    return _cached[key]


def _pmajor(a):
    """[C, N] -> [P, C//P, N] partition-major bf16, contiguous."""
    Cc, N = a.shape
    return np.ascontiguousarray(
        a.reshape(Cc // P, P, N).transpose(1, 0, 2).astype(BF))


def kernel(x, padding_mask, Wqkv, bqkv, Wproj, bproj):
    from concourse.bass_utils import run_bass_kernel_spmd

    x = np.asarray(x, dtype=np.float32)
    padding_mask = np.asarray(padding_mask)
    Wqkv = np.asarray(Wqkv, dtype=np.float32)
    bqkv = np.asarray(bqkv, dtype=np.float32)
    Wproj = np.asarray(Wproj, dtype=np.float32)
    bproj = np.asarray(bproj, dtype=np.float32)
    assert x.shape == (B, T, C), x.shape

    TC, steps, NBP, pad_tiles = _attn_meta(padding_mask)
    TCT = TC // P
    nc = _get_nc(TC, steps, NBP, pad_tiles)

    in_maps = []
    per_batch = {}
    for b in range(B):
        idx = np.nonzero(padding_mask[b])[0]
        cnt = len(idx)
        key_pos = np.full(TC, 1 << 20, np.int64)
        key_pos[:cnt] = idx
        valid = np.zeros(TC, np.float32)
        valid[:cnt] = 1.0
        xkv = np.zeros((TC, C), np.float32)
        xkv[:cnt] = x[b][idx]
        masks = np.zeros((max(NBP, 1), P, 512), BF)
        for qt in range(NT):
            qpos = qt * 512 + np.arange(512)[None, :]
            for (ktc, q0, qe, mi) in steps[qt]:
                if mi >= 0:
                    kp = key_pos[ktc * P:(ktc + 1) * P][:, None]
                    masks[mi] = (kp <= qpos).astype(BF)
        per_batch[b] = (
            _pmajor(x[b].T),                                    # [P, CT, T]
            _pmajor(xkv.T),                                     # [P, CT, TC]
            np.ascontiguousarray(masks.transpose(1, 0, 2)),     # [P, NBP, 512]
            valid.reshape(TCT, P).T.astype(np.float32),         # [P, TCT]
        )

    for core in range(N_CORES):
        b, g = divmod(core, 2)
        sl = slice(g * GC, (g + 1) * GC)
        xT16, xkvT16, masks, validp = per_batch[b]
        bq = bqkv[0 * C:1 * C][sl]
        bk = bqkv[1 * C:2 * C][sl]
        bb = np.concatenate(
            [bq.reshape(MT, P).T, bk.reshape(MT, P).T, validp], axis=1)
        wq = _pmajor(Wqkv[:, 0 * C:1 * C][:, sl])   # [P, CT, GC]
        wk = _pmajor(Wqkv[:, 1 * C:2 * C][:, sl])
        in_maps.append({
            "xT": xT16,
            "xkv": xkvT16,
            # [P, CT, GC] -> [P, MT, CT, P] m-major stationary layout
            "wq": np.ascontiguousarray(
                wq.reshape(P, CT, MT, P).transpose(0, 2, 1, 3)),
            "wk": np.ascontiguousarray(
                wk.reshape(P, CT, MT, P).transpose(0, 2, 1, 3)),
            "wv": _pmajor(Wqkv[:, 2 * C:3 * C][:, sl]),
            "wp": _pmajor(Wproj[g * GC:(g + 1) * GC, :]),
            "bb": np.ascontiguousarray(bb),
            "bv": np.ascontiguousarray(bqkv[2 * C:3 * C][sl]),
            "msk": masks,
        })

    trace = bool(os.environ.get("BASS_KERNEL_TRACE"))
    res = run_bass_kernel_spmd(
        nc, in_maps, core_ids=list(range(N_CORES)), trace=trace)
    _cached["last_result"] = res

    out = np.empty((B, T, C), dtype=np.float32)
    for b in range(B):
        out[b] = res.results[2 * b]["out"] + res.results[2 * b + 1]["out"] + bproj
    return out


# revision 33
# speedup vs baseline: 1.0795x; 1.0795x over previous
"""Causal self-attention (B=4, T=2048, C=1024, H=16) on 8 Trainium2 NeuronCores.

Sharding: core = (batch b = core//2, head-group g = core%2, 8 heads each).

Key ideas over the dense formulation:
  - Padded keys (~50% of tokens) are compacted away on the host: K/V
    projections, S = K^T.T Q^T, exp and PV run only over the ~1024 valid
    keys per batch (padded to TC, a multiple of 128). Causality on the
    compacted index is still a contiguous prefix per query, so the k-tile
    loop bounds shrink; ragged tile edges are handled by host-precomputed
    bf16 {0,1} masks multiplied into P after the exp (only over the ragged
    column span; pad rows are killed by zeroing their V rows + ones column).
  - All matmul operands are bf16 (PSUM accumulation stays fp32): halves
    DMA/SBUF and makes LDWEIGHTS cheap enough to hide behind matmuls.
  - All DRAM inputs are host-pre-transposed to partition-major [128, ...]
    layouts so DMA descriptors are large contiguous lines.
  - Q^T projection for q-block nt is computed during attention block nt-1
    (only nt=0 in the projection phase): the early attention blocks are
    exp-latency-bound, and the filler matmuls keep the PE busy so the HAM
    clock gate never drops back to 1.2GHz mid-kernel.
  - Softmax normalize: DVE copy+reciprocal of the PSUM denominator row,
    gpsimd broadcast, DVE multiplies; out-projection of a q-block issues
    right after its last head-pair normalizes.

Host: per-batch token compaction, transposes, bf16 casts, ragged masks,
sums the two head-group partials per batch and adds bproj.
"""

import os
import sys

for _p in ("/opt/trn_rl_repo",):
    if _p not in sys.path:
        sys.path.append(_p)

import numpy as np
import ml_dtypes

B, T, C = 4, 2048, 1024
H, D = 16, 64
HPC = 8          # heads per core
GC = HPC * D     # 512 channels per core
N_CORES = 8
P = 128
NT = T // 512    # 4 q-blocks of 512
MT = GC // 128   # 4 m-tiles (head pairs)
CT = C // 128    # 8 contraction tiles

BF = ml_dtypes.bfloat16
_cached = {}


def _attn_meta(pm):
    """Compile-time step structure shared by all cores (union over batches).

    Returns (TC, steps, NBP, pad_tiles): steps[qt] = tuple of
    (ktc, q0, qe, mask_idx); columns [q0, qe) of the tile get the ragged
    causal mask multiplied in (mask_idx == -1: tile fully valid, no mask).
    Columns >= qe are fully causal-valid in every batch; pad rows beyond the
    valid count are killed by zeroing their Vp rows (incl. the ones column),
    so they never need masking. pad_tiles lists k-tiles with pad slots in
    some batch."""
    idx = [np.nonzero(pm[b])[0] for b in range(B)]
    cnt = [len(i) for i in idx]
    TCT = -(-max(cnt) // P)
    TC = TCT * P
    INF = 1 << 30
    first = np.full((B, TCT), INF, np.int64)
    last = np.full((B, TCT), -1, np.int64)
    haspad = np.zeros((B, TCT), bool)
    for b in range(B):
        for t in range(TCT):
            lo, hi = t * P, min((t + 1) * P, cnt[b])
            if lo < cnt[b]:
                first[b, t] = idx[b][lo]
                last[b, t] = idx[b][hi - 1]
            haspad[b, t] = (t + 1) * P > cnt[b]
    steps = []
    nmask = 0
    for qt in range(NT):
        row = []
        for ktc in range(TCT):
            fmin = int(first[:, ktc].min())
            if fmin >= (qt + 1) * 512:
                continue
            q0 = max(0, fmin - qt * 512) & ~7
            lmax = int(last[:, ktc].max())
            if lmax > qt * 512:
                qe = min(512, (lmax - qt * 512 + 1 + 7) & ~7)
                mi = nmask
                nmask += 1
            else:
                qe = q0
                mi = -1
            row.append((ktc, q0, qe, mi))
        steps.append(tuple(row))
    pad_tiles = tuple(t for t in range(TCT) if haspad[:, t].any())
    return TC, tuple(steps), nmask, pad_tiles


def _build(TC, steps, NBP, pad_tiles):
    import concourse.tile as tile
    from concourse import bacc, mybir
    import concourse.bass as bass

    TCT = TC // P
    f32 = mybir.dt.float32
    bf = mybir.dt.bfloat16
    AF = mybir.ActivationFunctionType
    ADD = mybir.AluOpType.add
    MUL = mybir.AluOpType.mult
    NBIAS = 2 * MT + TCT  # packed bq/8 | bk | valid

    nc = bacc.Bacc("TRN2", target_bir_lowering=False, debug=False)

    # all inputs partition-major, host-pre-transposed
    xT_d = nc.dram_tensor("xT", [P, CT, T], bf, kind="ExternalInput")
    xkv_d = nc.dram_tensor("xkv", [P, CT, TC], bf, kind="ExternalInput")
    wq_d = nc.dram_tensor("wq", [P, MT, CT, P], bf, kind="ExternalInput")
    wk_d = nc.dram_tensor("wk", [P, MT, CT, P], bf, kind="ExternalInput")
    wv_d = nc.dram_tensor("wv", [P, CT, GC], bf, kind="ExternalInput")
    wp_d = nc.dram_tensor("wp", [P, MT, C], bf, kind="ExternalInput")
    bb_d = nc.dram_tensor("bb", [P, NBIAS], f32, kind="ExternalInput")
    bv_d = nc.dram_tensor("bv", [GC], f32, kind="ExternalInput")
    msk_d = nc.dram_tensor("msk", [P, max(NBP, 1), 512], bf, kind="ExternalInput")
    out_d = nc.dram_tensor("out", [T, C], f32, kind="ExternalOutput")

    with tile.TileContext(nc) as tc:
        with tc.tile_pool(name="persist", bufs=1) as persist, \
             tc.tile_pool(name="ps", bufs=2, space="PSUM") as ps_pool, \
             tc.tile_pool(name="ppool", bufs=6) as ppool, \
             tc.tile_pool(name="ypool", bufs=2) as ypool, \
             tc.tile_pool(name="rpool", bufs=2) as rpool, \
             tc.tile_pool(name="bcpool", bufs=2) as bcpool, \
             tc.tile_pool(name="otpool", bufs=3) as otpool:
            QT = persist.tile([P, MT, T], bf, tag="QT")
            KT = persist.tile([P, MT, TC], bf, tag="KT")
            Vp = persist.tile([P, TCT, HPC, D + 1], bf, tag="Vp")
            MS = persist.tile([P, max(NBP, 1), 512], bf, tag="MS")
            xq_s = persist.tile([P, CT, T], bf, tag="xq")
            xkv_s = persist.tile([P, CT, TC], bf, tag="xkv")
            wq_s = persist.tile([P, MT, CT, P], bf, tag="wq")
            wk_s = persist.tile([P, MT, CT, P], bf, tag="wk")
            wv_s = persist.tile([P, CT, GC], bf, tag="wv")
            wp_s = persist.tile([P, MT, C], bf, tag="wp")
            bb_s = persist.tile([P, NBIAS], f32, tag="bb")
            bv_s = persist.tile([P, GC], f32, tag="bv")

            # --- DMAs, in the order compute consumes them ---------------
            nc.sync.dma_start(bb_s[:], bb_d[:])
            bv_ap = bass.AP(tensor=bv_d[:].tensor, offset=0, ap=[[0, P], [1, GC]])
            nc.sync.dma_start(bv_s[:], bv_ap)
            nc.sync.dma_start(wk_s[:, 0, :, :], wk_d[:, 0, :, :])
            nc.sync.dma_start(xkv_s[:, 0:2, :], xkv_d[:, 0:2, :])
            nc.sync.dma_start(wk_s[:, 1:MT, :, :], wk_d[:, 1:MT, :, :])
            for c2 in range(2, CT, 2):
                nc.sync.dma_start(xkv_s[:, c2:c2 + 2, :], xkv_d[:, c2:c2 + 2, :])
            for c2 in range(0, CT, 2):
                nc.sync.dma_start(wv_s[:, c2:c2 + 2, :], wv_d[:, c2:c2 + 2, :])
            for c2 in range(0, CT, 2):
                nc.sync.dma_start(xq_s[:, c2:c2 + 2, :], xT_d[:, c2:c2 + 2, :])
            nc.sync.dma_start(wq_s[:], wq_d[:])
            if NBP:
                nc.sync.dma_start(MS[:], msk_d[:])
            nc.sync.dma_start(wp_s[:], wp_d[:])

            # ones column of Vp (softmax denominator accumulator row)
            nc.vector.memset(Vp[:, :, :, D:D + 1], 1.0)

            # matmul PSUM output is capped at one bank = 512 fp32/partition
            k_chunks = [(c0, min(c0 + 512, TC)) for c0 in range(0, TC, 512)]

            # ---------------- K^T projection ----------------------------
            for m in range(MT):
                for (c0, c1) in k_chunks:
                    sl = slice(c0, c1)
                    pk = ps_pool.tile([P, 512], f32, tag="SS", name="pk")
                    for c in range(CT):
                        nc.tensor.matmul(
                            pk[:, 0:c1 - c0], wk_s[:, m, c, :],
                            xkv_s[:, c, sl],
                            start=(c == 0), stop=(c == CT - 1))
                    # ScalarE evac: DVE stays free for attention-phase work
                    nc.scalar.activation(
                        KT[:, m, sl], pk[:, 0:c1 - c0], AF.Identity,
                        bias=bb_s[:, MT + m:MT + m + 1])

            # ---------------- V projection ------------------------------
            for tt in range(TCT):
                pv = ps_pool.tile([P, GC], f32, tag="SS", name="pv")
                for c in range(CT):
                    nc.tensor.matmul(
                        pv[:], xkv_s[:, c, tt * P:(tt + 1) * P], wv_s[:, c, :],
                        start=(c == 0), stop=(c == CT - 1))
                nc.vector.tensor_add(
                    Vp[:, tt, :, 0:D],
                    pv[:].rearrange("p (h d) -> p h d", h=HPC),
                    bv_s[:].rearrange("p (h d) -> p h d", h=HPC))
                if tt in pad_tiles:
                    # zero V rows + ones column of pad slots: they then
                    # contribute nothing to numerator or denominator
                    nc.vector.tensor_scalar(
                        out=Vp[:, tt, :, :], in0=Vp[:, tt, :, :],
                        scalar1=bb_s[:, 2 * MT + tt:2 * MT + tt + 1],
                        scalar2=None, op0=MUL)

            # ------------- Q^T projection (scaled by 1/8) ---------------
            def q_proj_tile(m, nt):
                sl = slice(nt * 512, (nt + 1) * 512)
                pq = ps_pool.tile([P, 512], f32, tag="SS", name="pq")
                for c in range(CT):
                    nc.tensor.matmul(
                        pq[:], wq_s[:, m, c, :], xq_s[:, c, sl],
                        start=(c == 0), stop=(c == CT - 1))
                # out = pq*0.125 + bq/8 on ScalarE (host pre-scales the bias)
                nc.scalar.activation(
                    QT[:, m, sl], pq[:], AF.Identity,
                    bias=bb_s[:, m:m + 1], scale=0.125)

            for m in range(MT):
                q_proj_tile(m, 0)  # nt >= 1 interleaved into attention

            # ---------------- attention + out-projection ----------------
            def proj_piece(yT_, tt, ts):
                for nh in range(2):
                    pp_ = ps_pool.tile([P, 512], f32, tag="OO")
                    for cj in range(MT):
                        nc.tensor.matmul(
                            pp_[:], yT_[:, cj, ts * P:(ts + 1) * P],
                            wp_s[:, cj, nh * 512:(nh + 1) * 512],
                            start=(cj == 0), stop=(cj == MT - 1))
                    ot = otpool.tile([P, 512], f32, tag="ot")
                    if nh == 0:
                        nc.vector.tensor_copy(ot[:], pp_[:])
                    else:
                        nc.scalar.activation(ot[:], pp_[:], AF.Copy)
                    nc.sync.dma_start(
                        out_d[tt * P:(tt + 1) * P, nh * 512:(nh + 1) * 512], ot[:])

            yTq_prev = None
            for qt in range(NT):
                yTq = ypool.tile([P, MT, 512], bf, tag="yTq")
                srow = steps[qt]
                ns = len(srow)
                OO_map = {}
                pend = []
                LAG = 3

                def normalize_and_aux(j_, qt=qt, yTq=yTq, yTq_prev=yTq_prev):
                    OO_ = OO_map[j_]
                    lraw = rpool.tile([1, 2, 512], f32, tag="lraw")
                    nc.vector.tensor_copy(lraw[0:1, :, :], OO_[D:D + 1, :, :])
                    rec = rpool.tile([1, 2, 512], f32, tag="rec")
                    nc.vector.reciprocal_approx_fast(
                        rec[0:1, :, :], lraw[0:1, :, :])
                    bc = bcpool.tile([P, 2, 512], f32, tag="bc")
                    nc.gpsimd.partition_broadcast(bc[:], rec[0:1, :, :], channels=P)
                    nc.vector.tensor_mul(yTq[0:D, j_, :], OO_[0:D, 0, :], bc[0:D, 0, :])
                    nc.vector.tensor_mul(yTq[D:P, j_, :], OO_[0:D, 1, :], bc[D:P, 1, :])
                    if yTq_prev is not None:
                        proj_piece(yTq_prev, (qt - 1) * 4 + j_, j_)

                def emit_pv(entry):
                    j_, si_, z_, PP_ = entry
                    OO_ = OO_map[j_]
                    ktc = srow[si_][0]
                    last_ = (si_ == ns - 1)
                    for e in range(2):
                        nc.tensor.matmul(
                            OO_[:, e, z_:512], Vp[:, ktc, 2 * j_ + e, :],
                            PP_[:, e, z_:512],
                            start=(si_ == 0), stop=last_)
                    if last_:
                        normalize_and_aux(j_)

                for j in range(MT):
                    OO_map[j] = ps_pool.tile(
                        [D + 1, 2, 512], f32, tag="OO", name="OO")
                    for si, (ktc, q0, qe, mi) in enumerate(srow):
                        SS = ps_pool.tile([P, 2, 512], f32, tag="SS")
                        nc.tensor.matmul(
                            SS[:, 0, q0:512], KT[0:D, j, ktc * P:(ktc + 1) * P],
                            QT[0:D, j, qt * 512 + q0:(qt + 1) * 512],
                            start=True, stop=True)
                        nc.tensor.matmul(
                            SS[:, 1, q0:512], KT[D:P, j, ktc * P:(ktc + 1) * P],
                            QT[D:P, j, qt * 512 + q0:(qt + 1) * 512],
                            start=True, stop=True)
                        PP = ppool.tile([P, 2, 512], bf, tag="PP")
                        nc.scalar.activation(
                            PP[:, :, q0:512], SS[:, :, q0:512], AF.Exp)
                        if mi >= 0:
                            tm = MS[:, mi, q0:qe]
                            mask_b = bass.AP(
                                tensor=tm.tensor, offset=tm.offset,
                                ap=[list(tm.ap[0]), [0, 2], list(tm.ap[1])])
                            nc.vector.tensor_mul(
                                PP[:, :, q0:qe], PP[:, :, q0:qe], mask_b)
                        pend.append((j, si, q0, PP))
                        if len(pend) > LAG:
                            emit_pv(pend.pop(0))
                    if qt < NT - 1:
                        # PE filler while exp drains: Q^T proj for the next
                        # q-block, one m-tile per head-pair iteration
                        q_proj_tile(j, qt + 1)
                while pend:
                    emit_pv(pend.pop(0))
                yTq_prev = yTq
            for ts in range(4):
                proj_piece(yTq_prev, (NT - 1) * 4 + ts, ts)

    nc.compile()
    return nc


def _get_nc(TC, steps, NBP, pad_tiles):
    key = (TC, steps, pad_tiles)
    if key not in _cached:
        _cached[key] = _build(TC, steps, NBP, pad_tiles)
    return _cached[key]


def _pmajor(a):
    """[C, N] -> [P, C//P, N] partition-major bf16, contiguous."""
    Cc, N = a.shape
    return np.ascontiguousarray(
        a.reshape(Cc // P, P, N).transpose(1, 0, 2).astype(BF))


def kernel(x, padding_mask, Wqkv, bqkv, Wproj, bproj):
    from concourse.bass_utils import run_bass_kernel_spmd

    x = np.asarray(x, dtype=np.float32)
    padding_mask = np.asarray(padding_mask)
    Wqkv = np.asarray(Wqkv, dtype=np.float32)
    bqkv = np.asarray(bqkv, dtype=np.float32)
    Wproj = np.asarray(Wproj, dtype=np.float32)
    bproj = np.asarray(bproj, dtype=np.float32)
    assert x.shape == (B, T, C), x.shape

    TC, steps, NBP, pad_tiles = _attn_meta(padding_mask)
    TCT = TC // P
    nc = _get_nc(TC, steps, NBP, pad_tiles)

    in_maps = []
    per_batch = {}
    for b in range(B):
        idx = np.nonzero(padding_mask[b])[0]
        cnt = len(idx)
        key_pos = np.full(TC, 1 << 20, np.int64)
        key_pos[:cnt] = idx
        valid = np.zeros(TC, np.float32)
        valid[:cnt] = 1.0
        xkv = np.zeros((TC, C), np.float32)
        xkv[:cnt] = x[b][idx]
        masks = np.zeros((max(NBP, 1), P, 512), BF)
        for qt in range(NT):
            qpos = qt * 512 + np.arange(512)[None, :]
            for (ktc, q0, qe, mi) in steps[qt]:
                if mi >= 0:
                    kp = key_pos[ktc * P:(ktc + 1) * P][:, None]
                    masks[mi] = (kp <= qpos).astype(BF)
        per_batch[b] = (
            _pmajor(x[b].T),                                    # [P, CT, T]
            _pmajor(xkv.T),                                     # [P, CT, TC]
            np.ascontiguousarray(masks.transpose(1, 0, 2)),     # [P, NBP, 512]
            valid.reshape(TCT, P).T.astype(np.float32),         # [P, TCT]
        )

    for core in range(N_CORES):
        b, g = divmod(core, 2)
        sl = slice(g * GC, (g + 1) * GC)
        xT16, xkvT16, masks, validp = per_batch[b]
        bq8 = bqkv[0 * C:1 * C][sl] * 0.125
        bk = bqkv[1 * C:2 * C][sl]
        bb = np.concatenate(
            [bq8.reshape(MT, P).T, bk.reshape(MT, P).T, validp], axis=1)
        wq = _pmajor(Wqkv[:, 0 * C:1 * C][:, sl])   # [P, CT, GC]
        wk = _pmajor(Wqkv[:, 1 * C:2 * C][:, sl])
        in_maps.append({
            "xT": xT16,
            "xkv": xkvT16,
            # [P, CT, GC] -> [P, MT, CT, P] m-major stationary layout
            "wq": np.ascontiguousarray(
                wq.reshape(P, CT, MT, P).transpose(0, 2, 1, 3)),
            "wk": np.ascontiguousarray(
                wk.reshape(P, CT, MT, P).transpose(0, 2, 1, 3)),
            "wv": _pmajor(Wqkv[:, 2 * C:3 * C][:, sl]),
            "wp": _pmajor(Wproj[g * GC:(g + 1) * GC, :]),
            "bb": np.ascontiguousarray(bb),
            "bv": np.ascontiguousarray(bqkv[2 * C:3 * C][sl]),
            "msk": masks,
        })

    trace = bool(os.environ.get("BASS_KERNEL_TRACE"))
    res = run_bass_kernel_spmd(
        nc, in_maps, core_ids=list(range(N_CORES)), trace=trace)
    _cached["last_result"] = res

    out = np.empty((B, T, C), dtype=np.float32)
    for b in range(B):
        out[b] = res.results[2 * b]["out"] + res.results[2 * b + 1]["out"] + bproj
    return out


# revision 35
# speedup vs baseline: 1.1069x; 1.0254x over previous
"""Causal self-attention (B=4, T=2048, C=1024, H=16) on 8 Trainium2 NeuronCores.

Sharding: core = (batch b = core//2, head-group g = core%2, 8 heads each).

Key ideas over the dense formulation:
  - Padded keys (~50% of tokens) are compacted away on the host: K/V
    projections, S = K^T.T Q^T, exp and PV run only over the ~1024 valid
    keys per batch (padded to TC, a multiple of 128). Causality on the
    compacted index is still a contiguous prefix per query, so the k-tile
    loop bounds shrink; ragged tile edges are handled by host-precomputed
    bf16 {0,1} masks multiplied into P after the exp (only over the ragged
    column span; pad rows are killed by zeroing their V rows + ones column).
  - All matmul operands are bf16 (PSUM accumulation stays fp32): halves
    DMA/SBUF and makes LDWEIGHTS cheap enough to hide behind matmuls.
  - All DRAM inputs are host-pre-transposed to partition-major [128, ...]
    layouts so DMA descriptors are large contiguous lines.
  - Q^T projection for q-block nt is computed during attention block nt-1
    (only nt=0 in the projection phase): the early attention blocks are
    exp-latency-bound, and the filler matmuls keep the PE busy so the HAM
    clock gate never drops back to 1.2GHz mid-kernel.
  - Softmax normalize: DVE copy+reciprocal of the PSUM denominator row,
    gpsimd broadcast, DVE multiplies; out-projection of a q-block issues
    right after its last head-pair normalizes.

Host: per-batch token compaction, transposes, bf16 casts, ragged masks,
sums the two head-group partials per batch and adds bproj.
"""

import os
import sys

for _p in ("/opt/trn_rl_repo",):
    if _p not in sys.path:
        sys.path.append(_p)

import numpy as np
import ml_dtypes

B, T, C = 4, 2048, 1024
H, D = 16, 64
HPC = 8          # heads per core
GC = HPC * D     # 512 channels per core
N_CORES = 8
P = 128
NT = T // 512    # 4 q-blocks of 512
MT = GC // 128   # 4 m-tiles (head pairs)
CT = C // 128    # 8 contraction tiles

BF = ml_dtypes.bfloat16
_cached = {}


def _attn_meta(pm):
    """Compile-time step structure shared by all cores (union over batches).

    Returns (TC, steps, NBP, pad_tiles): steps[qt] = tuple of
    (ktc, q0, qe, mask_idx); columns [q0, qe) of the tile get the ragged
    causal mask multiplied in (mask_idx == -1: tile fully valid, no mask).
    Columns >= qe are fully causal-valid in every batch; pad rows beyond the
    valid count are killed by zeroing their Vp rows (incl. the ones column),
    so they never need masking. pad_tiles lists k-tiles with pad slots in
    some batch."""
    idx = [np.nonzero(pm[b])[0] for b in range(B)]
    cnt = [len(i) for i in idx]
    TCT = -(-max(cnt) // P)
    TC = TCT * P
    INF = 1 << 30
    first = np.full((B, TCT), INF, np.int64)
    last = np.full((B, TCT), -1, np.int64)
    haspad = np.zeros((B, TCT), bool)
    for b in range(B):
        for t in range(TCT):
            lo, hi = t * P, min((t + 1) * P, cnt[b])
            if lo < cnt[b]:
                first[b, t] = idx[b][lo]
                last[b, t] = idx[b][hi - 1]
            haspad[b, t] = (t + 1) * P > cnt[b]
    steps = []
    nmask = 0
    for qt in range(NT):
        row = []
        for ktc in range(TCT):
            fmin = int(first[:, ktc].min())
            if fmin >= (qt + 1) * 512:
                continue
            q0 = max(0, fmin - qt * 512) & ~7
            lmax = int(last[:, ktc].max())
            if lmax > qt * 512:
                qe = min(512, (lmax - qt * 512 + 1 + 7) & ~7)
                mi = nmask
                nmask += 1
            else:
                qe = q0
                mi = -1
            row.append((ktc, q0, qe, mi))
        steps.append(tuple(row))
    pad_tiles = tuple(t for t in range(TCT) if haspad[:, t].any())
    return TC, tuple(steps), nmask, pad_tiles


def _build(TC, steps, NBP, pad_tiles):
    import concourse.tile as tile
    from concourse import bacc, mybir
    import concourse.bass as bass

    TCT = TC // P
    f32 = mybir.dt.float32
    bf = mybir.dt.bfloat16
    AF = mybir.ActivationFunctionType
    ADD = mybir.AluOpType.add
    MUL = mybir.AluOpType.mult
    NBIAS = 2 * MT + TCT  # packed bq/8 | bk | valid

    nc = bacc.Bacc("TRN2", target_bir_lowering=False, debug=False)

    # all inputs partition-major, host-pre-transposed
    xT_d = nc.dram_tensor("xT", [P, CT, T], bf, kind="ExternalInput")
    xkv_d = nc.dram_tensor("xkv", [P, CT, TC], bf, kind="ExternalInput")
    wq_d = nc.dram_tensor("wq", [P, MT, CT, P], bf, kind="ExternalInput")
    wk_d = nc.dram_tensor("wk", [P, MT, CT, P], bf, kind="ExternalInput")
    wv_d = nc.dram_tensor("wv", [P, CT, GC], bf, kind="ExternalInput")
    wp_d = nc.dram_tensor("wp", [P, MT, C], bf, kind="ExternalInput")
    bb_d = nc.dram_tensor("bb", [P, NBIAS], f32, kind="ExternalInput")
    bv_d = nc.dram_tensor("bv", [GC], f32, kind="ExternalInput")
    msk_d = nc.dram_tensor("msk", [P, max(NBP, 1), 512], bf, kind="ExternalInput")
    out_d = nc.dram_tensor("out", [T, C], f32, kind="ExternalOutput")

    with tile.TileContext(nc) as tc:
        with tc.tile_pool(name="persist", bufs=1) as persist, \
             tc.tile_pool(name="ps", bufs=2, space="PSUM") as ps_pool, \
             tc.tile_pool(name="ppool", bufs=6) as ppool, \
             tc.tile_pool(name="ypool", bufs=2) as ypool, \
             tc.tile_pool(name="rpool", bufs=2) as rpool, \
             tc.tile_pool(name="bcpool", bufs=2) as bcpool, \
             tc.tile_pool(name="otpool", bufs=3) as otpool:
            QT = persist.tile([P, MT, T], bf, tag="QT")
            KT = persist.tile([P, MT, TC], bf, tag="KT")
            Vp = persist.tile([P, TCT, HPC, D + 1], bf, tag="Vp")
            MS = persist.tile([P, max(NBP, 1), 512], bf, tag="MS")
            xq_s = persist.tile([P, CT, T], bf, tag="xq")
            xkv_s = persist.tile([P, CT, TC], bf, tag="xkv")
            wq_s = persist.tile([P, MT, CT, P], bf, tag="wq")
            wk_s = persist.tile([P, MT, CT, P], bf, tag="wk")
            wv_s = persist.tile([P, CT, GC], bf, tag="wv")
            wp_s = persist.tile([P, MT, C], bf, tag="wp")
            bb_s = persist.tile([P, NBIAS], f32, tag="bb")
            bv_s = persist.tile([P, GC], f32, tag="bv")

            # --- DMAs: few big transfers (sync-engine issue is ~600ns per
            # DMA instruction, so batching beats chunking) ---------------
            nc.sync.dma_start(wk_s[:], wk_d[:])
            nc.sync.dma_start(xkv_s[:, 0:4, :], xkv_d[:, 0:4, :])
            nc.sync.dma_start(xkv_s[:, 4:CT, :], xkv_d[:, 4:CT, :])
            nc.sync.dma_start(bb_s[:], bb_d[:])
            bv_ap = bass.AP(tensor=bv_d[:].tensor, offset=0, ap=[[0, P], [1, GC]])
            nc.sync.dma_start(bv_s[:], bv_ap)
            nc.sync.dma_start(wv_s[:], wv_d[:])
            nc.sync.dma_start(xq_s[:, 0:4, :], xT_d[:, 0:4, :])
            nc.sync.dma_start(xq_s[:, 4:CT, :], xT_d[:, 4:CT, :])
            nc.sync.dma_start(wq_s[:], wq_d[:])
            if NBP:
                nc.sync.dma_start(MS[:], msk_d[:])
            nc.sync.dma_start(wp_s[:], wp_d[:])

            # ones column of Vp (softmax denominator accumulator row)
            nc.vector.memset(Vp[:, :, :, D:D + 1], 1.0)

            # matmul PSUM output is capped at one bank = 512 fp32/partition
            k_chunks = [(c0, min(c0 + 512, TC)) for c0 in range(0, TC, 512)]

            # ---------------- K^T projection ----------------------------
            for m in range(MT):
                for (c0, c1) in k_chunks:
                    sl = slice(c0, c1)
                    pk = ps_pool.tile([P, 512], f32, tag="SS", name="pk")
                    for c in range(CT):
                        nc.tensor.matmul(
                            pk[:, 0:c1 - c0], wk_s[:, m, c, :],
                            xkv_s[:, c, sl],
                            start=(c == 0), stop=(c == CT - 1))
                    # ScalarE evac: DVE stays free for attention-phase work
                    nc.scalar.activation(
                        KT[:, m, sl], pk[:, 0:c1 - c0], AF.Identity,
                        bias=bb_s[:, MT + m:MT + m + 1])

            # ---------------- V projection ------------------------------
            for tt in range(TCT):
                pv = ps_pool.tile([P, GC], f32, tag="SS", name="pv")
                for c in range(CT):
                    nc.tensor.matmul(
                        pv[:], xkv_s[:, c, tt * P:(tt + 1) * P], wv_s[:, c, :],
                        start=(c == 0), stop=(c == CT - 1))
                nc.vector.tensor_add(
                    Vp[:, tt, :, 0:D],
                    pv[:].rearrange("p (h d) -> p h d", h=HPC),
                    bv_s[:].rearrange("p (h d) -> p h d", h=HPC))
                if tt in pad_tiles:
                    # zero V rows + ones column of pad slots: they then
                    # contribute nothing to numerator or denominator
                    nc.vector.tensor_scalar(
                        out=Vp[:, tt, :, :], in0=Vp[:, tt, :, :],
                        scalar1=bb_s[:, 2 * MT + tt:2 * MT + tt + 1],
                        scalar2=None, op0=MUL)

            # ------------- Q^T projection (scaled by 1/8) ---------------
            def q_proj_tile(m, nt):
                sl = slice(nt * 512, (nt + 1) * 512)
                pq = ps_pool.tile([P, 512], f32, tag="SS", name="pq")
                for c in range(CT):
                    nc.tensor.matmul(
                        pq[:], wq_s[:, m, c, :], xq_s[:, c, sl],
                        start=(c == 0), stop=(c == CT - 1))
                # out = pq*0.125 + bq/8 on ScalarE (host pre-scales the bias)
                nc.scalar.activation(
                    QT[:, m, sl], pq[:], AF.Identity,
                    bias=bb_s[:, m:m + 1], scale=0.125)

            for m in range(MT):
                q_proj_tile(m, 0)  # nt >= 1 interleaved into attention

            # ---------------- attention + out-projection ----------------
            def proj_piece(yT_, tt, ts):
                # both output halves in one 2-bank PSUM tile: one evac copy
                # (alternating DVE/ScalarE) and one row-contiguous DMA out
                pp_ = ps_pool.tile([P, 2, 512], f32, tag="OO", name="prj")
                for nh in range(2):
                    for cj in range(MT):
                        nc.tensor.matmul(
                            pp_[:, nh, :], yT_[:, cj, ts * P:(ts + 1) * P],
                            wp_s[:, cj, nh * 512:(nh + 1) * 512],
                            start=(cj == 0), stop=(cj == MT - 1))
                ot = otpool.tile([P, 2, 512], f32, tag="ot")
                if ts % 2 == 0:
                    nc.vector.tensor_copy(ot[:], pp_[:])
                else:
                    nc.scalar.activation(ot[:], pp_[:], AF.Copy)
                nc.sync.dma_start(out_d[tt * P:(tt + 1) * P, :], ot[:])

            yTq_prev = None
            for qt in range(NT):
                yTq = ypool.tile([P, MT, 512], bf, tag="yTq")
                srow = steps[qt]
                ns = len(srow)
                OO_map = {}
                pend = []
                LAG = 3

                def normalize_and_aux(j_, qt=qt, yTq=yTq, yTq_prev=yTq_prev):
                    OO_ = OO_map[j_]
                    lraw = rpool.tile([1, 2, 512], f32, tag="lraw")
                    nc.vector.tensor_copy(lraw[0:1, :, :], OO_[D:D + 1, :, :])
                    rec = rpool.tile([1, 2, 512], f32, tag="rec")
                    nc.vector.reciprocal_approx_fast(
                        rec[0:1, :, :], lraw[0:1, :, :])
                    bc = bcpool.tile([P, 2, 512], f32, tag="bc")
                    nc.gpsimd.partition_broadcast(bc[:], rec[0:1, :, :], channels=P)
                    nc.vector.tensor_mul(yTq[0:D, j_, :], OO_[0:D, 0, :], bc[0:D, 0, :])
                    nc.vector.tensor_mul(yTq[D:P, j_, :], OO_[0:D, 1, :], bc[D:P, 1, :])
                    if yTq_prev is not None:
                        proj_piece(yTq_prev, (qt - 1) * 4 + j_, j_)

                def emit_pv(entry):
                    j_, si_, z_, PP_ = entry
                    OO_ = OO_map[j_]
                    ktc = srow[si_][0]
                    last_ = (si_ == ns - 1)
                    for e in range(2):
                        nc.tensor.matmul(
                            OO_[:, e, z_:512], Vp[:, ktc, 2 * j_ + e, :],
                            PP_[:, e, z_:512],
                            start=(si_ == 0), stop=last_)
                    if last_:
                        normalize_and_aux(j_)

                for j in range(MT):
                    OO_map[j] = ps_pool.tile(
                        [D + 1, 2, 512], f32, tag="OO", name="OO")
                    for si, (ktc, q0, qe, mi) in enumerate(srow):
                        SS = ps_pool.tile([P, 2, 512], f32, tag="SS")
                        nc.tensor.matmul(
                            SS[:, 0, q0:512], KT[0:D, j, ktc * P:(ktc + 1) * P],
                            QT[0:D, j, qt * 512 + q0:(qt + 1) * 512],
                            start=True, stop=True)
                        nc.tensor.matmul(
                            SS[:, 1, q0:512], KT[D:P, j, ktc * P:(ktc + 1) * P],
                            QT[D:P, j, qt * 512 + q0:(qt + 1) * 512],
                            start=True, stop=True)
                        PP = ppool.tile([P, 2, 512], bf, tag="PP")
                        nc.scalar.activation(
                            PP[:, :, q0:512], SS[:, :, q0:512], AF.Exp)
                        if mi >= 0:
                            tm = MS[:, mi, q0:qe]
                            mask_b = bass.AP(
                                tensor=tm.tensor, offset=tm.offset,
                                ap=[list(tm.ap[0]), [0, 2], list(tm.ap[1])])
                            nc.vector.tensor_mul(
                                PP[:, :, q0:qe], PP[:, :, q0:qe], mask_b)
                        pend.append((j, si, q0, PP))
                        if len(pend) > LAG:
                            emit_pv(pend.pop(0))
                    if qt < NT - 1:
                        # PE filler while exp drains: Q^T proj for the next
                        # q-block, one m-tile per head-pair iteration
                        q_proj_tile(j, qt + 1)
                while pend:
                    emit_pv(pend.pop(0))
                yTq_prev = yTq
            for ts in range(4):
                proj_piece(yTq_prev, (NT - 1) * 4 + ts, ts)

    nc.compile()
    return nc


def _get_nc(TC, steps, NBP, pad_tiles):
    key = (TC, steps, pad_tiles)
    if key not in _cached:
        _cached[key] = _build(TC, steps, NBP, pad_tiles)
    return _cached[key]


def _pmajor(a):
    """[C, N] -> [P, C//P, N] partition-major bf16, contiguous."""
    Cc, N = a.shape
    return np.ascontiguousarray(
        a.reshape(Cc // P, P, N).transpose(1, 0, 2).astype(BF))


def kernel(x, padding_mask, Wqkv, bqkv, Wproj, bproj):
    from concourse.bass_utils import run_bass_kernel_spmd

    x = np.asarray(x, dtype=np.float32)
    padding_mask = np.asarray(padding_mask)
    Wqkv = np.asarray(Wqkv, dtype=np.float32)
    bqkv = np.asarray(bqkv, dtype=np.float32)
    Wproj = np.asarray(Wproj, dtype=np.float32)
    bproj = np.asarray(bproj, dtype=np.float32)
    assert x.shape == (B, T, C), x.shape

    TC, steps, NBP, pad_tiles = _attn_meta(padding_mask)
    TCT = TC // P
    nc = _get_nc(TC, steps, NBP, pad_tiles)

    in_maps = []
    per_batch = {}
    for b in range(B):
        idx = np.nonzero(padding_mask[b])[0]
        cnt = len(idx)
        key_pos = np.full(TC, 1 << 20, np.int64)
        key_pos[:cnt] = idx
        valid = np.zeros(TC, np.float32)
        valid[:cnt] = 1.0
        xkv = np.zeros((TC, C), np.float32)
        xkv[:cnt] = x[b][idx]
        masks = np.zeros((max(NBP, 1), P, 512), BF)
        for qt in range(NT):
            qpos = qt * 512 + np.arange(512)[None, :]
            for (ktc, q0, qe, mi) in steps[qt]:
                if mi >= 0:
                    kp = key_pos[ktc * P:(ktc + 1) * P][:, None]
                    masks[mi] = (kp <= qpos).astype(BF)
        per_batch[b] = (
            _pmajor(x[b].T),                                    # [P, CT, T]
            _pmajor(xkv.T),                                     # [P, CT, TC]
            np.ascontiguousarray(masks.transpose(1, 0, 2)),     # [P, NBP, 512]
            valid.reshape(TCT, P).T.astype(np.float32),         # [P, TCT]
        )

    for core in range(N_CORES):
        b, g = divmod(core, 2)
        sl = slice(g * GC, (g + 1) * GC)
        xT16, xkvT16, masks, validp = per_batch[b]
        bq8 = bqkv[0 * C:1 * C][sl] * 0.125
        bk = bqkv[1 * C:2 * C][sl]
        bb = np.concatenate(
            [bq8.reshape(MT, P).T, bk.reshape(MT, P).T, validp], axis=1)
        wq = _pmajor(Wqkv[:, 0 * C:1 * C][:, sl])   # [P, CT, GC]
        wk = _pmajor(Wqkv[:, 1 * C:2 * C][:, sl])
        in_maps.append({
            "xT": xT16,
            "xkv": xkvT16,
            # [P, CT, GC] -> [P, MT, CT, P] m-major stationary layout
            "wq": np.ascontiguousarray(
                wq.reshape(P, CT, MT, P).transpose(0, 2, 1, 3)),
            "wk": np.ascontiguousarray(
                wk.reshape(P, CT, MT, P).transpose(0, 2, 1, 3)),
            "wv": _pmajor(Wqkv[:, 2 * C:3 * C][:, sl]),
            "wp": _pmajor(Wproj[g * GC:(g + 1) * GC, :]),
            "bb": np.ascontiguousarray(bb),
            "bv": np.ascontiguousarray(bqkv[2 * C:3 * C][sl]),
            "msk": masks,
        })

    trace = bool(os.environ.get("BASS_KERNEL_TRACE"))
    res = run_bass_kernel_spmd(
        nc, in_maps, core_ids=list(range(N_CORES)), trace=trace)
    _cached["last_result"] = res

    out = np.empty((B, T, C), dtype=np.float32)
    for b in range(B):
        out[b] = res.results[2 * b]["out"] + res.results[2 * b + 1]["out"] + bproj
    return out
